# revision 2
# baseline (speedup 1.0000x reference)
"""Trainium2 Bass kernel for FeatureAugmentationNetwork2.

Reference computation (N=M=8192, H=512, tau=1, c=0.5):
    q = features @ Wq.T + bq
    k = memory_features @ Wk.T + bk
    attn = softmax(q @ k.T, axis=-1)
    out = c * features + (1-c) * attn @ memory_features

Sharding: features (queries) split across 8 cores on the N axis;
memory_features / weights replicated.  Each core computes its
[1024, 8192] attention slab independently; outputs are concatenated.

Algebraic restructuring (exact):
  - bk adds a per-row constant to the logits -> softmax-invariant -> dropped.
  - S = q @ k.T = (features @ W2 + b2) @ memory.T
    with W2 = Wq.T @ Wk (computed on-chip in f32r), b2 = bq @ Wk.
  - softmax without a row max: exp(s - C) with fixed C = 100 (bf16/f32
    exponent range absorbs the spread; denominators renormalize).
  - E_T ([m, n] layout) feeds attn.V as lhsT without any attention-matrix
    transpose; the softmax denominator is fused into the AV matmuls by
    storing V rows as [V(512) | 1]: av2 covers cols [256:513] so the ones
    column lands in av2's PSUM col 256 (aug col 512).

Differences vs the 332us baseline (329 -> target ~265us):
  - memT (S stationary operand) is produced by DMA XBAR transposes of the
    bf16 V tile instead of 256 PE transpose instructions: the PE stream in
    the main loop is pure matmuls.
  - S runs bf16 x bf16 (q2T cast to bf16); q2 itself is still computed in
    the f32/f32r pipeline.  Simulated end-to-end rel err ~8.8e-3 (< 2e-2).
  - mv ring (32 of 64 m-tiles resident) with the ones column at the END of
    the row, so DMA-transpose sources are 32B-aligned.
  - ~40 warmup matmuls on a memset tile un-throttle the PE HAM clock gate
    during the initial DMA wait.
  - feat is pre-scaled by c in the preamble and the final merge is a single
    scalar_tensor_tensor, shortening the end-of-kernel critical chain.
"""

from contextlib import ExitStack

import numpy as np

import concourse.bass as bass
import concourse.tile as tile
from concourse import bacc, mybir
from concourse.alu_op_type import AluOpType
from concourse.bass_utils import run_bass_kernel_spmd
from concourse.masks import make_identity

N_CORES = 8
N, M, H = 8192, 8192, 512
N_LOC = N // N_CORES  # 1024 query rows per core
C_OFF = 100.0  # fixed softmax exp offset
MERGE = 0.5

F32 = mybir.dt.float32
F32R = mybir.dt.float32r
BF16 = mybir.dt.bfloat16

MVW = 528  # mv row: [V(512) | ones | 15 pad] -> 1056B rows, 32B aligned
MV_RING = 32  # m-tiles resident (2 rounds deep)


def _emit(nc, tc, ctx, d):
    NT = N_LOC // 128  # 8  query-row tiles
    MT = M // 128  # 64 memory-row tiles
    HC = H // 128  # 4  feature-dim chunks
    GROUP = 16  # memory tiles per AV accumulation round
    NH = N_LOC // 512  # 2  n halves (512-wide matmul free dim)
    DMA_MT = 4  # memory tiles per load
    n_rounds = MT // GROUP

    main_sb = ctx.enter_context(tc.tile_pool(name="main_sb", bufs=1))
    ident = main_sb.tile([128, 128], F32)
    wum = main_sb.tile([128, 128], BF16)

    q2T = main_sb.tile([128, HC, N_LOC], BF16)
    bias_t = main_sb.tile([128, 1], F32)
    mv = main_sb.tile([128, MV_RING, MVW], BF16)
    aug = main_sb.tile([128, NT, 516], F32)  # col 512 holds the denominator
    rh = main_sb.tile([128, NT], F32)
    feat = main_sb.tile([128, NT, H], F32)

    raw_pool = ctx.enter_context(tc.tile_pool(name="raw", bufs=2))
    met_pool = ctx.enter_context(tc.tile_pool(name="met", bufs=2 * GROUP))
    mtp_ps = ctx.enter_context(tc.tile_pool(name="mtp", bufs=2, space="PSUM"))
    s_ps_pool = ctx.enter_context(tc.tile_pool(name="sps", bufs=2, space="PSUM"))
    av1_pool = ctx.enter_context(tc.tile_pool(name="av1", bufs=2, space="PSUM"))
    av2_pool = ctx.enter_context(tc.tile_pool(name="av2", bufs=2, space="PSUM"))

    # ---- PE warmup: un-throttle HAM during the initial DMA wait ----------
    nc.vector.memset(wum[:], 0.25)
    wu = mtp_ps.tile([128, 128], F32, tag="mtp", name="warm")
    N_WARM = 40
    for i in range(N_WARM):
        nc.tensor.matmul(wu[:], wum[:], wum[:], start=(i == 0), stop=(i == N_WARM - 1))

    nc.vector.memset(bias_t[:], -C_OFF)
    make_identity(nc, ident[:])
    # ones column for every mv ring slot (written once; casts leave it alone)
    nc.vector.memset(mv[:, :, H : H + 1], 1.0)

    def load_round(g):
        tiles = []
        for half in range(GROUP // DMA_MT):
            r = raw_pool.tile([128, DMA_MT, H], F32, tag="raw")
            base = (g * GROUP + half * DMA_MT) * 128
            nc.sync.dma_start(
                r[:],
                d["memory_features"][base : base + DMA_MT * 128, :].rearrange(
                    "(t p) h -> p t h", p=128
                ),
            )
            tiles.append(r)
        return tiles

    def prep_tile(raws, g, tl):
        """bf16 cast into the mv ring + 4 DMA XBAR transposes -> met (bf16)."""
        mt = g * GROUP + tl
        slot = mt % MV_RING
        raw = raws[tl // DMA_MT][:, tl % DMA_MT, :]
        nc.vector.tensor_copy(mv[:, slot, 0:H], raw)
        met = met_pool.tile([128, HC, 128], BF16, tag="met")
        for ic in range(HC):
            nc.sync.dma_start(
                met[:, ic, :],
                mv[:, slot, ic * 128 : (ic + 1) * 128],
                transpose=True,
            )
        return met

    # ---------------- preamble -------------------------------------------
    with tc.tile_pool(name="pre_keep", bufs=1) as pre_keep, ExitStack() as pre_ctx:
        pre_w = pre_ctx.enter_context(tc.tile_pool(name="pre_w", bufs=1))
        wq = pre_w.tile([128, HC, H], F32)
        wk = pre_w.tile([128, HC, H], F32)
        nc.sync.dma_start(wq[:], d["Wq"].rearrange("(c p) h -> p c h", p=128))
        nc.sync.dma_start(wk[:], d["Wk"].rearrange("(c p) h -> p c h", p=128))
        bq = pre_w.tile([128, HC], F32)
        nc.sync.dma_start(bq[:], d["bq"].rearrange("(c p) -> p c", p=128))
        for nt in range(NT):
            nc.sync.dma_start(
                feat[:, nt, :],
                d["features"][nt * 128 : (nt + 1) * 128, :],
            )
        raws0 = load_round(0)

        featT = pre_keep.tile([128, HC, N_LOC], F32R)

        def emit_featT(nt):
            fps = mtp_ps.tile([128, H], F32, tag="mtp", name=f"fps{nt}")
            for ic in range(HC):
                nc.tensor.transpose(
                    fps[:, ic * 128 : (ic + 1) * 128],
                    feat[:, nt, ic * 128 : (ic + 1) * 128],
                    ident[:],
                )
            nc.vector.tensor_copy(
                featT[:, :, nt * 128 : (nt + 1) * 128],
                fps[:].rearrange("p (c n) -> p c n", c=HC),
            )
            # pre-scale for the final merge (runs after the transpose reads)
            nc.scalar.mul(feat[:, nt, :], feat[:, nt, :], MERGE)

        emit_featT(0)

        # W2[i, j] = sum_o Wq[o, i] * Wk[o, j]   (f32r)
        wqr = pre_w.tile([128, HC, H], F32R)
        wkr = pre_w.tile([128, HC, H], F32R)
        nc.vector.tensor_copy(wqr[:], wq[:])
        nc.vector.tensor_copy(wkr[:], wk[:])
        w2r = pre_keep.tile([128, HC, H], F32R)
        for ic in range(HC):
            ps = mtp_ps.tile([128, H], F32, tag="mtp", name=f"w2ps{ic}")
            for oc in range(HC):
                nc.tensor.matmul(
                    ps[:],
                    wqr[:, oc, ic * 128 : (ic + 1) * 128],
                    wkr[:, oc, :],
                    start=(oc == 0),
                    stop=(oc == HC - 1),
                )
            nc.vector.tensor_copy(w2r[:, ic, :], ps[:])

        # b2T[j] = sum_o Wk[o, j] * bq[o]
        b2full = mtp_ps.tile([128, H], F32, tag="mtp", name="b2ps")
        b2ps = b2full[:, :HC]
        for jc in range(HC):
            for oc in range(HC):
                nc.tensor.matmul(
                    b2ps[:, jc : jc + 1],
                    wk[:, oc, jc * 128 : (jc + 1) * 128],
                    bq[:, oc : oc + 1],
                    start=(oc == 0),
                    stop=(oc == HC - 1),
                    skip_group_check=True,
                )
        b2t = pre_keep.tile([128, HC], F32)
        nc.vector.tensor_copy(b2t[:], b2ps)
        for nt in range(1, NT):
            emit_featT(nt)
        pre_ctx.close()  # release wq/wk/wqr/wkr/bq

        # q2T[j, n] = sum_i W2[i, j] featT[i, n] + b2T[j]  -> bf16
        for jc in range(HC):
            for nh in range(NH):
                ps = mtp_ps.tile([128, 512], F32, tag="mtp", name=f"q2ps{jc}_{nh}")
                for ic in range(HC):
                    nc.tensor.matmul(
                        ps[:],
                        w2r[:, ic, jc * 128 : (jc + 1) * 128],
                        featT[:, ic, nh * 512 : (nh + 1) * 512],
                        start=(ic == 0),
                        stop=(ic == HC - 1),
                    )
                nc.vector.tensor_scalar_add(
                    q2T[:, jc, nh * 512 : (nh + 1) * 512], ps[:], b2t[:, jc : jc + 1]
                )

        # round-0 mem prep last: the memory DMAs have had the preamble to land
        mets = [prep_tile(raws0, 0, tl) for tl in range(GROUP)]

    # ---------------- main loop over memory-tile rounds --------------------
    et_pool = ctx.enter_context(tc.tile_pool(name="et", bufs=GROUP + 4))
    out_pool = ctx.enter_context(tc.tile_pool(name="out_sb", bufs=2))
    ets = {}
    for g in range(n_rounds):
        if g + 1 < n_rounds:
            next_raws = load_round(g + 1)

        for tl in range(GROUP):
            mt = g * GROUP + tl
            met = mets[tl]
            # S_T[m-block, n] = sum_i memT[i, m] q2T[i, n]; E_T = exp(S_T - C)
            et = et_pool.tile([128, N_LOC], BF16, tag="et")
            for nh in range(NH):
                sp = s_ps_pool.tile([128, 512], F32, tag="sps")
                for ic in range(HC):
                    nc.tensor.matmul(
                        sp[:],
                        met[:, ic, :],
                        q2T[:, ic, nh * 512 : (nh + 1) * 512],
                        start=(ic == 0),
                        stop=(ic == HC - 1),
                    )
                nc.scalar.activation(
                    et[:, nh * 512 : (nh + 1) * 512],
                    sp[:],
                    mybir.ActivationFunctionType.Exp,
                    bias=bias_t[:],
                )
            ets[mt] = et
            if g + 1 < n_rounds:
                mets[tl] = prep_tile(next_raws, g + 1, tl)

        # AV + fused denominator: aug[n, 0:256] += E.T @ V_lo,
        # aug[n, 256:513] += E.T @ [V_hi | ones]  (den -> aug col 512)
        for nt in range(NT):
            av1 = av1_pool.tile([128, 256], F32, tag="av1")
            av2 = av2_pool.tile([128, 257], F32, tag="av2")
            for tl in range(GROUP):
                mt = g * GROUP + tl
                slot = mt % MV_RING
                eb = ets[mt][:, nt * 128 : (nt + 1) * 128]
                nc.tensor.matmul(
                    av1[:],
                    eb,
                    mv[:, slot, 0:256],
                    start=(tl == 0),
                    stop=(tl == GROUP - 1),
                )
                nc.tensor.matmul(
                    av2[:],
                    eb,
                    mv[:, slot, 256 : H + 1],
                    start=(tl == 0),
                    stop=(tl == GROUP - 1),
                )
            if g == 0:
                nc.vector.tensor_copy(aug[:, nt, 0:256], av1[:])
                nc.vector.tensor_copy(aug[:, nt, 256 : H + 1], av2[:])
            else:
                nc.vector.tensor_tensor(
                    aug[:, nt, 0:256], aug[:, nt, 0:256], av1[:], AluOpType.add
                )
                nc.vector.tensor_tensor(
                    aug[:, nt, 256 : H + 1],
                    aug[:, nt, 256 : H + 1],
                    av2[:],
                    AluOpType.add,
                )
            if g == n_rounds - 1:
                # denominator complete: out = aug * (1-c)/den + c*feat
                nc.vector.reciprocal(rh[:, nt : nt + 1], aug[:, nt, H : H + 1])
                nc.vector.tensor_scalar_mul(
                    rh[:, nt : nt + 1], rh[:, nt : nt + 1], 1.0 - MERGE
                )
                o = out_pool.tile([128, H], F32, tag="out")
                nc.vector.scalar_tensor_tensor(
                    o[:],
                    aug[:, nt, 0:H],
                    rh[:, nt : nt + 1],
                    feat[:, nt, :],
                    op0=AluOpType.mult,
                    op1=AluOpType.add,
                )
                nc.sync.dma_start(d["out"][nt * 128 : (nt + 1) * 128, :], o[:])


def build_module():
    nc = bacc.Bacc("TRN2", target_bir_lowering=False, debug=False)
    d = {
        "features": nc.dram_tensor("features", [N_LOC, H], F32, kind="ExternalInput").ap(),
        "memory_features": nc.dram_tensor(
            "memory_features", [M, H], F32, kind="ExternalInput"
        ).ap(),
        "Wq": nc.dram_tensor("Wq", [H, H], F32, kind="ExternalInput").ap(),
        "Wk": nc.dram_tensor("Wk", [H, H], F32, kind="ExternalInput").ap(),
        "bq": nc.dram_tensor("bq", [H], F32, kind="ExternalInput").ap(),
        "out": nc.dram_tensor("out", [N_LOC, H], F32, kind="ExternalOutput").ap(),
    }
    with tile.TileContext(nc) as tc, ExitStack() as ctx:
        _emit(nc, tc, ctx, d)
    nc.compile()
    return nc


_CACHED = None


def kernel(features, memory_features, Wq, bq, Wk, bk=None, **_ignored):
    global _CACHED
    if _CACHED is None:
        _CACHED = build_module()
    nc = _CACHED

    features = np.ascontiguousarray(np.asarray(features, dtype=np.float32))
    memory_features = np.ascontiguousarray(np.asarray(memory_features, dtype=np.float32))
    Wq = np.ascontiguousarray(np.asarray(Wq, dtype=np.float32))
    Wk = np.ascontiguousarray(np.asarray(Wk, dtype=np.float32))
    bq = np.ascontiguousarray(np.asarray(bq, dtype=np.float32))

    in_maps = []
    for c in range(N_CORES):
        in_maps.append(
            {
                "features": features[c * N_LOC : (c + 1) * N_LOC],
                "memory_features": memory_features,
                "Wq": Wq,
                "Wk": Wk,
                "bq": bq,
            }
        )
    res = run_bass_kernel_spmd(nc, in_maps, core_ids=list(range(N_CORES)))
    return np.concatenate([res.results[c]["out"] for c in range(N_CORES)], axis=0)
